# revision 2
# baseline (speedup 1.0000x reference)
"""Binary-weight dense layer on 8 TRN2 NeuronCores.

Computes out = x @ sign(W) + b for x:[8192,4096] f32, W:[4096,4096] f32,
b:[4096] f32, sharded row-wise over x (tensor-parallel over the batch dim:
each core computes a [1024, 4096] slice of the output; no collectives).

Per-core kernel strategy (single-pass bf16):
  - The host hands each core its x shard already transposed (xT:[4096,1024]
    f32, a pure layout transform) so the contraction dim lands on the SBUF
    partition axis with plain contiguous DMA loads — no on-device transpose.
  - xT streams HBM->SBUF through SWDGE DMAs that cast fp32->bf16 in the DMA
    datapath, building a resident lhsT block [K=128, 32*1024] per k-tile.
    sign(W) is exact in bf16 and x's bf16 rounding puts the output rel err
    at ~1.7e-3, well inside the 2e-2 gate, so one bf16 pass suffices.
  - W tiles stream fp32 on the HWDGE sync queue and are quantized on the
    scalar engine (Sign activation) into bf16 rhs tiles.
  - Main loop: for each n-slice (512 cols) keep 8 PSUM banks (one per m-tile)
    accumulating over all 32 k-tiles; evict with a DVE bias-add; output
    writes go out on the SWDGE queue so they never head-block the next
    slice's W loads on the sync queue.
"""

import sys

if "/opt/trn_rl_repo" not in sys.path:
    sys.path.insert(0, "/opt/trn_rl_repo")

import numpy as np

import concourse.bass as bass
import concourse.mybir as mybir
import concourse.tile as tile
from concourse import bacc
from concourse.bass_utils import run_bass_kernel_spmd

N_CORES = 8
P = 128

B, N_IN, N_UNITS = 8192, 4096, 4096
M_SH = B // N_CORES  # 1024 rows of x per core

F32 = mybir.dt.float32
BF16 = mybir.dt.bfloat16


def build_module(m_sh=M_SH, k_dim=N_IN, n_dim=N_UNITS, reps=1, timing=False):
    """Build + compile the per-core Bass module (same program on all cores).

    reps>1 wraps the whole pipeline in a hardware For_i loop and timing=True
    swaps the big output for an internal DRAM tensor plus a tiny sink output;
    both are used only for wall-clock timing calibration (the marginal cost of
    an extra rep is the kernel's HW exec time, free of host/tunnel overhead)."""
    nc = bacc.Bacc("TRN2", target_bir_lowering=False, debug=False)

    xt_in = nc.dram_tensor("xT", [k_dim, m_sh], F32, kind="ExternalInput")
    w_in = nc.dram_tensor("W", [k_dim, n_dim], F32, kind="ExternalInput")
    b_in = nc.dram_tensor("b", [n_dim], F32, kind="ExternalInput")
    if timing:
        out = nc.dram_tensor("out_scratch", [m_sh, n_dim], F32)
        sink = nc.dram_tensor("out", [P, 512], F32, kind="ExternalOutput")
    else:
        out = nc.dram_tensor("out", [m_sh, n_dim], F32, kind="ExternalOutput")

    NT = 512  # psum free dim (one bank of fp32)
    KT = P  # contraction tile
    m_tiles = m_sh // P
    k_tiles = k_dim // KT
    n_slices = n_dim // NT

    import contextlib

    with tile.TileContext(nc) as tc:
        with (
            tc.For_i(0, reps, 1) if reps > 1 else contextlib.nullcontext(),
            tc.tile_pool(name="xt", bufs=1) as xt_pool,
            tc.tile_pool(name="const", bufs=1) as const_pool,
            tc.tile_pool(name="wf", bufs=6) as wf_pool,
            tc.tile_pool(name="wq", bufs=6) as wq_pool,
            tc.tile_pool(name="psum", bufs=8, space="PSUM") as psum_pool,
            tc.tile_pool(name="osb", bufs=4) as out_pool,
        ):
            # SBUF-resident transposed activations: column block kt holds
            # [K=128, M=1024] for contraction tile kt (bf16, cast in-DMA).
            xt = xt_pool.tile([P, k_tiles * m_sh], BF16)

            b_bc = const_pool.tile([P, n_dim], F32)
            nc.sync.dma_start(
                b_bc[:], b_in.ap().rearrange("(a n) -> a n", a=1).broadcast_to([P, n_dim])
            )

            osb = None
            for ns in range(n_slices):
                nss = slice(ns * NT, (ns + 1) * NT)
                psums = [
                    psum_pool.tile([P, NT], F32, name=f"ps_{ns}_{mt}", tag="ps")
                    for mt in range(m_tiles)
                ]
                for kt in range(k_tiles):
                    if ns == 0:
                        # xT k-tile loads interleave with the first n-slice's
                        # W loads; SWDGE casts fp32->bf16 in the DMA datapath.
                        os_ = slice(kt * m_sh, (kt + 1) * m_sh)
                        nc.gpsimd.dma_start(
                            xt[:, os_], xt_in[kt * KT : (kt + 1) * KT, :]
                        )
                    wf = wf_pool.tile([P, NT], F32, name=f"wf_{ns}_{kt}", tag="wf")
                    nc.sync.dma_start(wf[:], w_in[kt * KT : (kt + 1) * KT, nss])
                    wq = wq_pool.tile([P, NT], BF16, name=f"wq_{ns}_{kt}", tag="wq")
                    nc.scalar.sign(wq[:], wf[:])
                    for mt in range(m_tiles):
                        o = kt * m_sh + mt * P
                        nc.tensor.matmul(
                            psums[mt][:],
                            xt[:, o : o + P],
                            wq[:],
                            start=(kt == 0),
                            stop=(kt == k_tiles - 1),
                        )
                for mt in range(m_tiles):
                    osb = out_pool.tile([P, NT], F32, name=f"osb_{ns}_{mt}", tag="osb")
                    nc.vector.tensor_add(osb[:], psums[mt][:], b_bc[:, nss])
                    nc.gpsimd.dma_start(out[mt * P : (mt + 1) * P, nss], osb[:])
            if timing:
                nc.sync.dma_start(sink[:], osb[:])

    nc.compile()
    return nc


_NC_CACHE = {}


def _get_module(m_sh=M_SH, k_dim=N_IN, n_dim=N_UNITS):
    key = (m_sh, k_dim, n_dim)
    if key not in _NC_CACHE:
        _NC_CACHE[key] = build_module(m_sh, k_dim, n_dim)
    return _NC_CACHE[key]


def make_in_maps(x, W, b):
    """Per-core input dicts: x batch-shard transposed to [k, m] layout."""
    xs = np.ascontiguousarray(
        np.transpose(x.reshape(N_CORES, M_SH, N_IN), (0, 2, 1))
    )
    return [{"xT": xs[i], "W": W, "b": b} for i in range(N_CORES)]


def kernel(x: np.ndarray, W: np.ndarray, b: np.ndarray) -> np.ndarray:
    x = np.ascontiguousarray(np.asarray(x, dtype=np.float32))
    W = np.ascontiguousarray(np.asarray(W, dtype=np.float32))
    b = np.ascontiguousarray(np.asarray(b, dtype=np.float32))
    assert x.shape == (B, N_IN) and W.shape == (N_IN, N_UNITS) and b.shape == (N_UNITS,)

    nc = _get_module()
    in_maps = make_in_maps(x, W, b)
    res = run_bass_kernel_spmd(nc, in_maps, core_ids=list(range(N_CORES)))
    return np.concatenate(
        [res.results[i]["out"] for i in range(N_CORES)], axis=0
    ).astype(np.float32)
